# revision 1
# baseline (speedup 1.0000x reference)
"""Trainium2 Bass kernel for nn_CAD_GCN (gnn_message_passing).

Math: with x [B,C,H,W], S = H*W, x_node = mean_s x,
  h   = x_node @ g1_w.T + g1_b
  z1  = h*g2_w + g2_b
  y   = sum_n (theta_w x + theta_b)[n] * z1[n]
      = sum_c w_eff[c]*x[c,s] + bias_eff          (no Bmap materialization)
  out = tanh(x + phi_w[c]*y + phi_b[c])
where w_eff = x_node @ A + r, bias_eff = x_node @ a + s0 with
  A = g2_w*(g1_w.T @ theta_w), r = (g2_w*g1_b + g2_b) @ theta_w
  a = g2_w*(g1_w.T @ theta_b), s0 = (g2_w*g1_b + g2_b) @ theta_b
(all host-precomputable from the tiny parameter tensors).

Sharding: pure data parallel, 2 samples per core on 8 cores. Each core
sees its x slice as [128, 65536] (partition p = (b_local, c)).

Device program per core:
  pass 1: stream x chunks, free-dim reduce -> sums [128,1]
  tiny:   w2 [128,2] = mbd.T @ sums_bd + rbd;  M2 = w2 @ phi2 [128,128]
          (per-sample rank-1 map), bias2 [128,1] via abd
  pass 2: per 512-col tile: z = M2.T @ x (one PE matmul, f32r),
          s = x + z (DVE), out = tanh(s + bias2) (ACT), DMA out.
The first RETAIN chunks stay resident in SBUF between the passes to cut
HBM re-read traffic; the big matmuls run in float32r (fast fp32 PE mode,
~1e-4 relative error vs ~2e-2-style gates).
"""

import sys

for _p in ("/opt/trn_rl_repo",):
    if _p not in sys.path:
        sys.path.insert(0, _p)

import numpy as np

import concourse.bacc as bacc
import concourse.bass as bass
import concourse.mybir as mybir
import concourse.tile as tile
from concourse.bass_utils import run_bass_kernel_spmd

F32 = mybir.dt.float32
F32R = mybir.dt.float32r

B, C, H, W = 16, 64, 256, 256
S = H * W                      # 65536 pixels per sample
NCORES = 8
BPC = B // NCORES              # 2 samples per core
P = BPC * C                    # 128 partitions = (b_local, c)

CHUNK = 2048                   # free-dim columns per DMA (1 MiB per chunk)
SUB = 512                      # matmul free-dim tile (one fp32 PSUM bank)
RETAIN = 18                    # chunks kept in SBUF between pass 1 and 2
USE_F32R = True                # fast fp32 matmul mode for the big matmuls

NCHUNK = S // CHUNK
NSUB = CHUNK // SUB
INV_S = 1.0 / float(S)


def _build_program(n_pix=S, chunk=CHUNK, retain=RETAIN, use_f32r=USE_F32R,
                   xstream_bufs=3, spool_bufs=2, opool_bufs=4,
                   psy_bufs=3, psz_bufs=3, store_eng="sync", load2_eng="gpsimd",
                   lc=None, sc=None, ypool_bufs=4, inplace=True, zwide=1, order_mode=0,
                   rl=2, load1_eng="sync", retload_eng="sync",
                   act_reduce=True):
    """chunk: compute granularity (DVE/ACT/PE tiling, and spool size).
    lc: load-chunk (DMA granularity for x, multiple of chunk). sc: store-chunk.
    retain counts load-chunks."""
    lc = lc or chunk
    sc = sc or chunk
    assert lc % chunk == 0 and sc % chunk == 0 and n_pix % lc == 0
    nload = n_pix // lc
    cpl = lc // chunk              # compute chunks per load chunk
    cps = sc // chunk              # compute chunks per store chunk
    nsub = chunk // SUB if chunk >= SUB else 1
    sub = min(SUB, chunk)

    nc = bacc.Bacc("TRN2", target_bir_lowering=False, debug=False)

    # f32r is bit-identical to f32; the PE's fast fp32 matmul path requires
    # its operands to be *typed* f32r end-to-end. DVE/ACT consumers bitcast
    # back to plain f32.
    XD = F32R if use_f32r else F32

    x_d = nc.dram_tensor("x", [P, n_pix], XD, kind="ExternalInput")
    mbd_d = nc.dram_tensor("mbd", [P, P], F32, kind="ExternalInput")
    abd_d = nc.dram_tensor("abd", [P, P], F32, kind="ExternalInput")
    rbd_d = nc.dram_tensor("rbd", [P, 2], F32, kind="ExternalInput")
    bcol_d = nc.dram_tensor("bcol", [P, 1], F32, kind="ExternalInput")
    phi2_d = nc.dram_tensor("phi2", [2, P], F32, kind="ExternalInput")
    eye_d = nc.dram_tensor("eye", [P, P], F32, kind="ExternalInput")
    out_d = nc.dram_tensor("out", [P, n_pix], F32, kind="ExternalOutput")

    X = mybir.AxisListType.X
    Tanh = mybir.ActivationFunctionType.Tanh

    def asf32(ap):
        return ap.bitcast(F32) if use_f32r else ap

    with tile.TileContext(nc) as tc:
        with (
            tc.tile_pool(name="consts", bufs=1) as consts,
            tc.tile_pool(name="xstream", bufs=xstream_bufs) as xpool,
            tc.tile_pool(name="xret", bufs=1) as rpool,
            tc.tile_pool(name="stats", bufs=1) as stats,
            tc.tile_pool(name="spool", bufs=spool_bufs) as spool,
            tc.tile_pool(name="opool", bufs=opool_bufs) as opool,
            tc.tile_pool(name="ps_small", bufs=1, space="PSUM") as ps_small,
            tc.tile_pool(name="ps_z", bufs=psz_bufs, space="PSUM") as ps_z,
        ):
            # ---- constants to SBUF ----
            mbd_sb = consts.tile([P, P], F32, name="mbd_sb")
            nc.gpsimd.dma_start(mbd_sb[:], mbd_d[:])
            abd_sb = consts.tile([P, P], F32, name="abd_sb")
            nc.gpsimd.dma_start(abd_sb[:], abd_d[:])
            rbd_sb = consts.tile([P, 2], F32, name="rbd_sb")
            nc.gpsimd.dma_start(rbd_sb[:], rbd_d[:])
            bcol_sb = consts.tile([P, 1], F32, name="bcol_sb")
            nc.gpsimd.dma_start(bcol_sb[:], bcol_d[:])
            phi2_sb = consts.tile([2, P], F32, name="phi2_sb")
            nc.gpsimd.dma_start(phi2_sb[:], phi2_d[:])
            eye_sb = consts.tile([P, P], F32, name="eye_sb")
            nc.gpsimd.dma_start(eye_sb[:], eye_d[:])

            # ---- pass 1: channel sums ----
            # Retained chunks live in one contiguous mega-tile, loaded with a
            # few multi-MB DMAs (HBM efficiency rises sharply with transfer
            # size); reduces still run per load-chunk on subtile slices.
            sums_nk = stats.tile([P, nload], F32, name="sums_nk")
            xret = rpool.tile([P, retain * lc], XD, name="xret") if retain else None
            for g0 in range(0, retain, rl):
                g1 = min(g0 + rl, retain)
                getattr(nc, retload_eng).dma_start(
                    xret[:, g0 * lc : g1 * lc], x_d[:, g0 * lc : g1 * lc]
                )
            Copy = mybir.ActivationFunctionType.Copy
            xtiles = []
            for i in range(nload):
                if i < retain:
                    xt = xret[:, i * lc : (i + 1) * lc]
                else:
                    xt = xpool.tile([P, lc], XD, name="xs", tag="xs")
                    getattr(nc, load1_eng).dma_start(
                        xt[:], x_d[:, i * lc : (i + 1) * lc]
                    )
                if act_reduce and i >= retain and i % 2 == 1:
                    # balance pass-1 reductions across DVE and ACT: the
                    # activation computes the free-dim sum via accum_out; the
                    # copy output is written in place (streamed tiles have no
                    # other consumers in pass 1, so this costs no SBUF)
                    nc.scalar.activation(
                        asf32(xt[:]), asf32(xt[:]), Copy,
                        accum_out=sums_nk[:, i : i + 1],
                    )
                else:
                    nc.vector.reduce_sum(sums_nk[:, i : i + 1], asf32(xt[:]), X)
                xtiles.append(xt if i < retain else None)

            sums = stats.tile([P, 1], F32, name="sums")
            nc.vector.reduce_sum(sums[:, 0:1], sums_nk[:], X)

            # block-diagonal copy of sums: col j holds sample j's sums
            sums_bd = stats.tile([P, 2], F32, name="sums_bd")
            nc.vector.memset(sums_bd[:], 0.0)
            nc.vector.tensor_copy(sums_bd[0:C, 0:1], sums[0:C, 0:1])
            nc.vector.tensor_copy(sums_bd[C:P, 1:2], sums[C:P, 0:1])

            # ---- tiny stage: w2 [P,2] and bias2 [P,1] ----
            w2_ps = ps_small.tile([P, 2], F32, name="w2_ps", tag="tiny")
            nc.tensor.matmul(w2_ps[:], mbd_sb[:], sums_bd[:], start=True, stop=True)
            w2_sb = stats.tile([P, 2], F32, name="w2_sb")
            nc.vector.tensor_add(w2_sb[:], w2_ps[:], rbd_sb[:])

            # M2 = w2 @ phi2  [P, P]: per-sample rank-1 map so that
            # z = M2.T @ x directly (one matmul per tile in pass 2)
            w2T_ps = ps_small.tile([2, P], F32, name="w2T_ps", tag="tiny2")
            nc.tensor.transpose(w2T_ps[:], w2_sb[:], eye_sb[:])
            w2T_sb = stats.tile([2, P], F32, name="w2T_sb")
            nc.scalar.copy(w2T_sb[:], w2T_ps[:])
            M2_ps = ps_small.tile([P, P], F32, name="M2_ps", tag="tiny3")
            nc.tensor.matmul(M2_ps[:], w2T_sb[:], phi2_sb[:], start=True, stop=True)
            M2_r = stats.tile([P, P], XD, name="M2_r")
            nc.vector.tensor_copy(M2_r[:], M2_ps[:])

            b2_ps = ps_small.tile([P, 2], F32, name="b2_ps", tag="tiny")
            nc.tensor.matmul(b2_ps[:], abd_sb[:], sums_bd[:], start=True, stop=True)
            b2_tmp = stats.tile([P, 1], F32, name="b2_tmp")
            nc.vector.reduce_sum(b2_tmp[:, 0:1], b2_ps[:], X)
            bias2 = stats.tile([P, 1], F32, name="bias2")
            nc.vector.tensor_add(bias2[:], b2_tmp[:], bcol_sb[:])

            # ---- pass 2 ----
            ncomp = n_pix // chunk
            ot = None
            # streamed chunks first: their loads overlap the tiny stage, and
            # the kernel tail lands on SBUF-resident retained chunks.
            # order_mode mixes some retained chunks into the streamed phase to
            # smooth the transition.
            streamed = [i for i in range(ncomp) if i // cpl >= retain]
            retained = [i for i in range(ncomp) if i // cpl < retain]
            if order_mode == 0:
                order = streamed + retained
            else:
                order = []
                a = b = 0
                while a < len(streamed) or b < len(retained):
                    for _ in range(order_mode):
                        if a < len(streamed):
                            order.append(streamed[a]); a += 1
                    if b < len(retained):
                        order.append(retained[b]); b += 1
            for i in order:
                li, lj = divmod(i, cpl)            # load-chunk index / offset
                if lj == 0:
                    if li < retain:
                        xt = xtiles[li]
                    else:
                        xt = xpool.tile([P, lc], XD, name="xs", tag="xs")
                        getattr(nc, load2_eng).dma_start(
                            xt[:], x_d[:, li * lc : (li + 1) * lc]
                        )
                si, sj = divmod(i, cps)
                if sj == 0:
                    ot = opool.tile([P, sc], F32, name="ot", tag="ot")
                if inplace:
                    st = ot
                    soff = sj * chunk
                else:
                    st = spool.tile([P, chunk], F32, name="st", tag="st")
                    soff = 0
                # zwide: one PSUM tile spanning `zwide` banks; matmuls fill
                # 512-wide bank-aligned slices, one DVE add covers them all
                zw = sub * zwide
                for jz in range(max(1, chunk // zw)):
                    z_ps = ps_z.tile([P, zw], F32, name="z_ps", tag="z")
                    for j in range(zwide):
                        off = jz * zw + j * sub
                        gsl = slice(lj * chunk + off, lj * chunk + off + sub)
                        nc.tensor.matmul(
                            z_ps[:, j * sub : (j + 1) * sub], M2_r[:], xt[:, gsl],
                            start=True, stop=True,
                        )
                    gz = slice(lj * chunk + jz * zw, lj * chunk + (jz + 1) * zw)
                    nc.vector.tensor_add(
                        st[:, soff + jz * zw : soff + (jz + 1) * zw],
                        asf32(xt[:, gz]), z_ps[:],
                    )
                nc.scalar.activation(
                    ot[:, sj * chunk : (sj + 1) * chunk],
                    st[:, soff : soff + chunk], Tanh,
                    bias=bias2[:, 0:1],
                )
                if sj == cps - 1:
                    getattr(nc, store_eng).dma_start(
                        out_d[:, si * sc : (si + 1) * sc], ot[:]
                    )

    nc.compile()
    return nc


def _host_consts(theta_w, theta_b, g1_w, g1_b, g2_w, g2_b, phi_w, phi_b):
    """Fold the GCN parameter chain into the device-side constant tensors."""
    f8 = np.float64
    theta_w = theta_w.astype(f8)
    theta_b = theta_b.astype(f8)
    g1_w = g1_w.astype(f8)
    g1_b = g1_b.astype(f8)
    g2w = f8(g2_w.reshape(-1)[0])
    g2b = f8(g2_b.reshape(-1)[0])
    phi_w = phi_w.astype(f8)
    phi_b = phi_b.astype(f8)

    # w_eff = x_node @ A + r ; bias_eff = x_node @ a + s0
    A = g2w * (g1_w.T @ theta_w)            # [C, C]
    r = (g2w * g1_b + g2b) @ theta_w        # [C]
    a = g2w * (g1_w.T @ theta_b)            # [C]
    s0 = (g2w * g1_b + g2b) @ theta_b       # scalar

    # mbd[p', p] = ind(b(p')==b(p)) * A[c(p'), c(p)] / S
    mbd = np.zeros((P, P), f8)
    mbd[0:C, 0:C] = A * INV_S
    mbd[C:P, C:P] = A * INV_S
    # abd[p', p] = ind(b(p')==b(p)) * phi_w[c(p)] * a[c(p')] / S
    abd = np.zeros((P, P), f8)
    abd[0:C, 0:C] = np.outer(a, phi_w) * INV_S
    abd[C:P, C:P] = np.outer(a, phi_w) * INV_S
    # rbd[p, j] = ind(b(p)==j) * r[c(p)]
    rbd = np.zeros((P, 2), f8)
    rbd[0:C, 0] = r
    rbd[C:P, 1] = r
    # bcol[p] = phi_w[c]*s0 + phi_b[c]
    bcol = np.tile(phi_w * s0 + phi_b, BPC)[:, None]
    # phi2[j, p] = ind(b(p)==j) * phi_w[c(p)]
    phi2 = np.zeros((2, P), f8)
    phi2[0, 0:C] = phi_w
    phi2[1, C:P] = phi_w

    c32 = lambda t: np.ascontiguousarray(t, dtype=np.float32)
    return {
        "mbd": c32(mbd),
        "abd": c32(abd),
        "rbd": c32(rbd),
        "bcol": c32(bcol),
        "phi2": c32(phi2),
        "eye": c32(np.eye(P)),
    }


_NC_CACHE = {}


def _get_nc():
    key = (S, CHUNK, RETAIN, USE_F32R)
    if key not in _NC_CACHE:
        _NC_CACHE[key] = _build_program(S, CHUNK, RETAIN, USE_F32R)
    return _NC_CACHE[key]


def _run(inputs, trace=False):
    x = np.ascontiguousarray(np.asarray(inputs["x"]), dtype=np.float32)
    consts = _host_consts(
        np.asarray(inputs["theta_w"]), np.asarray(inputs["theta_b"]),
        np.asarray(inputs["g1_w"]), np.asarray(inputs["g1_b"]),
        np.asarray(inputs["g2_w"]), np.asarray(inputs["g2_b"]),
        np.asarray(inputs["phi_w"]), np.asarray(inputs["phi_b"]),
    )
    in_maps = []
    for k in range(NCORES):
        xk = x[k * BPC : (k + 1) * BPC].reshape(P, S)
        in_maps.append({"x": np.ascontiguousarray(xk), **consts})

    nc = _get_nc()
    res = run_bass_kernel_spmd(
        nc, in_maps, core_ids=list(range(NCORES)), trace=trace
    )
    out = np.empty((B, C, H, W), dtype=np.float32)
    for k in range(NCORES):
        out[k * BPC : (k + 1) * BPC] = res.results[k]["out"].reshape(BPC, C, H, W)
    return out, res


def kernel(**inputs):
    out, _ = _run(inputs, trace=False)
    return out



# revision 2
# speedup vs baseline: 1.1537x; 1.1537x over previous
"""Trainium2 Bass kernel for nn_CAD_GCN (gnn_message_passing), v2.1.

Same structure as v2 (fp16-resident x, fused (I+M2) matmul, ACT tanh)
with tail/transition optimizations:
  - pass-1 tail ladder: the final load chunks shrink (2048/1024/512/512)
    and alternate ACT copy+accum / DVE-direct-reduce so the last-chunk
    reduce is short; the DVE accumulator is reduced early (after its
    last 4096 chunk, which arrives well before the stream ends).
  - tiny-stage matmuls run in bf16 (f32 costs 4 cycles/row on the PE).
  - pass-2 tail ladder: the final columns use smaller ACT tiles and
    store chunks so the end-of-kernel serial chain (last ACT -> last
    store -> sem) is short.
"""

import sys

for _p in ("/opt/trn_rl_repo",):
    if _p not in sys.path:
        sys.path.insert(0, _p)

import numpy as np
import ml_dtypes

import concourse.bacc as bacc
import concourse.mybir as mybir
import concourse.tile as tile
from concourse.bass_utils import run_bass_kernel_spmd

F32 = mybir.dt.float32
F16 = mybir.dt.float16
BF16 = mybir.dt.bfloat16

B, C, H, W = 16, 64, 256, 256
S = H * W
NCORES = 8
BPC = B // NCORES
P = BPC * C
INV_S = 1.0 / float(S)

ACC_W = 2048                   # pass-1 DVE accumulator width
ZT = 2048                      # pass-2 PSUM/ACT tile cols (4 banks)
MMT = 512                      # matmul free-dim tile (1 fp32 PSUM bank)

# pass-1 load chunks: (cols, sink). "dve" -> adds into acc (two ACC_W
# halves), "act" -> in-place Copy+accum_out, "dver" -> DVE direct
# reduce_sum (cheap for small tails).
LOADS = (
    [(4096, "dve"), (4096, "act")] * 6      # c0..c11
    + [(4096, "dve")]                        # c12 (last acc chunk)
    + [(4096, "act")]                        # c13
    + [(2048, "act"), (2048, "dver"), (1024, "act"), (1024, "dver"),
       (512, "act"), (512, "dver"), (512, "act"), (512, "dver")]
)
assert sum(c for c, _ in LOADS) == S

# pass-2 column plan: (store_cols, [act_tile_cols...]) per store chunk
STORES = [(4096, [2048, 2048])] * 15 + [
    (2048, [2048]),
    (1024, [1024]),
    (1024, [1024]),
]
assert sum(sc for sc, _ in STORES) == S


def _build_program():
    nc = bacc.Bacc("TRN2", target_bir_lowering=False, debug=False)

    x_d = nc.dram_tensor("x", [P, S], F16, kind="ExternalInput")
    mbd_d = nc.dram_tensor("mbd", [P, P], BF16, kind="ExternalInput")
    abd_d = nc.dram_tensor("abd", [P, P], BF16, kind="ExternalInput")
    rbdT_d = nc.dram_tensor("rbdT", [2, P], F32, kind="ExternalInput")
    bcol_d = nc.dram_tensor("bcol", [P, 1], F32, kind="ExternalInput")
    phi2_d = nc.dram_tensor("phi2", [2, P], BF16, kind="ExternalInput")
    eye_d = nc.dram_tensor("eye", [P, P], F32, kind="ExternalInput")
    out_d = nc.dram_tensor("out", [P, S], F16, kind="ExternalOutput")

    X = mybir.AxisListType.X
    Tanh = mybir.ActivationFunctionType.Tanh
    Copy = mybir.ActivationFunctionType.Copy

    with tile.TileContext(nc) as tc:
        with (
            tc.tile_pool(name="consts", bufs=1) as consts,
            tc.tile_pool(name="xret", bufs=1) as rpool,
            tc.tile_pool(name="stats", bufs=1) as stats,
            tc.tile_pool(name="opool", bufs=5) as opool,
            tc.tile_pool(name="ps", bufs=2, space="PSUM") as ps,
        ):
            # ---- constants to SBUF (SWDGE so SP stays free for x loads) ----
            mbd_sb = consts.tile([P, P], BF16, name="mbd_sb")
            nc.gpsimd.dma_start(mbd_sb[:], mbd_d[:])
            abd_sb = consts.tile([P, P], BF16, name="abd_sb")
            nc.gpsimd.dma_start(abd_sb[:], abd_d[:])
            rbdT_sb = consts.tile([2, P], F32, name="rbdT_sb")
            nc.gpsimd.dma_start(rbdT_sb[:], rbdT_d[:])
            bcol_sb = consts.tile([P, 1], F32, name="bcol_sb")
            nc.gpsimd.dma_start(bcol_sb[:], bcol_d[:])
            phi2_sb = consts.tile([2, P], BF16, name="phi2_sb")
            nc.gpsimd.dma_start(phi2_sb[:], phi2_d[:])
            eye_sb = consts.tile([P, P], F32, name="eye_sb")
            nc.gpsimd.dma_start(eye_sb[:], eye_d[:])

            # ---- pass 1: loads + channel sums ----
            xret = rpool.tile([P, S], F16, name="xret")
            acc = stats.tile([P, ACC_W], F16, name="acc")
            nc.vector.memset(acc[:], 0.0)
            sums_nk = stats.tile([P, len(LOADS) + 2], F32, name="sums_nk")

            nacc = 0
            off = 0
            n_dve_acc = sum(1 for _, s in LOADS if s == "dve")
            seen_dve = 0
            for cols, sink in LOADS:
                sl = slice(off, off + cols)
                nc.sync.dma_start(xret[:, sl], x_d[:, sl])
                if sink == "dve":
                    for j in range(max(1, cols // ACC_W)):
                        g0 = off + j * ACC_W
                        g1 = off + min((j + 1) * ACC_W, cols)
                        nc.vector.tensor_add(
                            acc[:, 0 : g1 - g0], acc[:, 0 : g1 - g0],
                            xret[:, g0:g1],
                        )
                    seen_dve += 1
                    if seen_dve == n_dve_acc:
                        nc.vector.reduce_sum(sums_nk[:, nacc : nacc + 1], acc[:], X)
                        nacc += 1
                elif sink == "act":
                    nc.scalar.activation(
                        xret[:, sl], xret[:, sl], Copy,
                        accum_out=sums_nk[:, nacc : nacc + 1],
                    )
                    nacc += 1
                else:  # dver
                    nc.vector.reduce_sum(sums_nk[:, nacc : nacc + 1], xret[:, sl], X)
                    nacc += 1
                off += cols

            sums = stats.tile([P, 1], F32, name="sums")
            nc.vector.reduce_sum(sums[:, 0:1], sums_nk[:, 0:nacc], X)

            # block-diagonal copy of sums: col j holds sample j's sums
            sums_bd = stats.tile([P, 2], BF16, name="sums_bd")
            nc.vector.memset(sums_bd[:], 0.0)
            nc.vector.tensor_copy(sums_bd[0:C, 0:1], sums[0:C, 0:1])
            nc.vector.tensor_copy(sums_bd[C:P, 1:2], sums[C:P, 0:1])

            # ---- tiny stage (bf16 matmuls: 1 cycle/row on the PE) ----
            # w2T [2, P] = sums_bd^T @ mbd  (+ rbdT)
            t0 = ps.tile([P, ZT], F32, name="t0", tag="z")
            w2T_ps = t0[0:2, 0:P]
            nc.tensor.matmul(w2T_ps, sums_bd[:], mbd_sb[:], start=True, stop=True)
            w2T_sb = stats.tile([2, P], BF16, name="w2T_sb")
            nc.vector.tensor_add(w2T_sb[:], w2T_ps, rbdT_sb[:])

            # M2p = (w2T @ phi2) + I, cast fp16
            t1 = ps.tile([P, ZT], F32, name="t1", tag="z")
            M2_ps = t1[:, 0:P]
            nc.tensor.matmul(M2_ps, w2T_sb[:], phi2_sb[:], start=True, stop=True)
            M2p = stats.tile([P, P], F16, name="M2p")
            nc.vector.tensor_add(M2p[:], M2_ps, eye_sb[:])

            # bias2 [P,1] = reduce(abd^T @ sums_bd) + bcol
            t2 = ps.tile([P, ZT], F32, name="t2", tag="z")
            b2_ps = t2[:, 0:2]
            nc.tensor.matmul(b2_ps, abd_sb[:], sums_bd[:], start=True, stop=True)
            b2_tmp = stats.tile([P, 1], F32, name="b2_tmp")
            nc.vector.reduce_sum(b2_tmp[:, 0:1], b2_ps, X)
            bias2 = stats.tile([P, 1], F32, name="bias2")
            nc.vector.tensor_add(bias2[:], b2_tmp[:], bcol_sb[:])

            # ---- pass 2 ----
            off = 0
            for sc, tiles in STORES:
                ot = opool.tile([P, sc], F16, name="ot", tag="ot")
                toff = 0
                for tcols in tiles:
                    z = ps.tile([P, ZT], F32, name="z", tag="z")
                    for j in range(tcols // MMT):
                        g0 = off + toff + j * MMT
                        nc.tensor.matmul(
                            z[:, j * MMT : (j + 1) * MMT], M2p[:],
                            xret[:, g0 : g0 + MMT],
                            start=True, stop=True,
                        )
                    nc.scalar.activation(
                        ot[:, toff : toff + tcols], z[:, 0:tcols], Tanh,
                        bias=bias2[:, 0:1],
                    )
                    toff += tcols
                nc.sync.dma_start(out_d[:, off : off + sc], ot[:])
                off += sc

    nc.compile()
    return nc


def _host_consts(theta_w, theta_b, g1_w, g1_b, g2_w, g2_b, phi_w, phi_b):
    """Fold the GCN parameter chain into the device-side constant tensors."""
    f8 = np.float64
    theta_w = theta_w.astype(f8)
    theta_b = theta_b.astype(f8)
    g1_w = g1_w.astype(f8)
    g1_b = g1_b.astype(f8)
    g2w = f8(g2_w.reshape(-1)[0])
    g2b = f8(g2_b.reshape(-1)[0])
    phi_w = phi_w.astype(f8)
    phi_b = phi_b.astype(f8)

    A = g2w * (g1_w.T @ theta_w)            # [C, C]
    r = (g2w * g1_b + g2b) @ theta_w        # [C]
    a = g2w * (g1_w.T @ theta_b)            # [C]
    s0 = (g2w * g1_b + g2b) @ theta_b       # scalar

    mbd = np.zeros((P, P), f8)
    mbd[0:C, 0:C] = A * INV_S
    mbd[C:P, C:P] = A * INV_S
    abd = np.zeros((P, P), f8)
    abd[0:C, 0:C] = np.outer(a, phi_w) * INV_S
    abd[C:P, C:P] = np.outer(a, phi_w) * INV_S
    rbdT = np.zeros((2, P), f8)
    rbdT[0, 0:C] = r
    rbdT[1, C:P] = r
    bcol = np.tile(phi_w * s0 + phi_b, BPC)[:, None]
    phi2 = np.zeros((2, P), f8)
    phi2[0, 0:C] = phi_w
    phi2[1, C:P] = phi_w

    c32 = lambda t: np.ascontiguousarray(t, dtype=np.float32)
    cb = lambda t: np.ascontiguousarray(t.astype(np.float32)).astype(
        ml_dtypes.bfloat16
    )
    return {
        "mbd": cb(mbd),
        "abd": cb(abd),
        "rbdT": c32(rbdT),
        "bcol": c32(bcol),
        "phi2": cb(phi2),
        "eye": c32(np.eye(P)),
    }


_NC_CACHE = {}


def _get_nc():
    if "nc" not in _NC_CACHE:
        _NC_CACHE["nc"] = _build_program()
    return _NC_CACHE["nc"]


def _run(inputs, trace=False):
    x = np.asarray(inputs["x"])
    consts = _host_consts(
        np.asarray(inputs["theta_w"]), np.asarray(inputs["theta_b"]),
        np.asarray(inputs["g1_w"]), np.asarray(inputs["g1_b"]),
        np.asarray(inputs["g2_w"]), np.asarray(inputs["g2_b"]),
        np.asarray(inputs["phi_w"]), np.asarray(inputs["phi_b"]),
    )
    x16 = np.ascontiguousarray(x, dtype=np.float16).reshape(NCORES, P, S)
    in_maps = [{"x": x16[k], **consts} for k in range(NCORES)]

    nc = _get_nc()
    res = run_bass_kernel_spmd(
        nc, in_maps, core_ids=list(range(NCORES)), trace=trace
    )
    out = np.empty((B, C, H, W), dtype=np.float32)
    for k in range(NCORES):
        out[k * BPC : (k + 1) * BPC] = (
            res.results[k]["out"].astype(np.float32).reshape(BPC, C, H, W)
        )
    return out, res


def kernel(**inputs):
    out, _ = _run(inputs, trace=False)
    return out


# revision 3
# speedup vs baseline: 1.1711x; 1.0151x over previous
"""Trainium2 Bass kernel for nn_CAD_GCN (gnn_message_passing), v4.

Math (per core, 2 samples): out = tanh(x + phi_w*y + phi_b) with
y = sum_c w_eff[c] x[c,:] + bias_eff, w_eff = x_node@A + r (see
_host_consts); x_node is the per-sample spatial mean.

v4 layout: the host packs each sample's pixels onto the FULL 128
partitions: partition p = (pixel_half h, channel c), columns =
32768 half-pixels; per-core tensor [128, 65536] = [sample0 | sample1].
The two samples are processed as two phases:
  phase A: stream sample-0's 32768 cols (23 us of DMA), reduce per
    chunk (DVE fp16 adds / ACT copy+accum / DVE direct for tails),
    fold the (h,c) partial sums over h with a tiny PE matmul, build
    M2p0 = blockdiag(outer(w_eff,phi_w)+I, same) and bias2; then run
    sample-0's matmul+tanh+store pass WHILE sample-1's columns are
    still streaming (phase B loads queue right behind phase A's).
  phase B: same for sample 1; its reductions run on DVE under
    sample-0's tanh stream, so ACT (the only tanh engine and the
    critical resource, 0.833ns/col) stays continuously busy.
All compute tiles are full 128-partition width (engine cost is per
free-dim column, so half-height tiles would double engine time).

x is staged fp16 by the host; output written fp16, host upcasts
(rel-err gate 2e-2; measured ~7e-4).
"""

import sys

for _p in ("/opt/trn_rl_repo",):
    if _p not in sys.path:
        sys.path.insert(0, _p)

import numpy as np
import ml_dtypes

import concourse.bacc as bacc
import concourse.mybir as mybir
import concourse.tile as tile
from concourse.bass_utils import run_bass_kernel_spmd

F32 = mybir.dt.float32
F16 = mybir.dt.float16
BF16 = mybir.dt.bfloat16

B, C, H, W = 16, 64, 256, 256
S = H * W                      # pixels per sample
HS = S // 2                    # half-pixels = cols per phase
NCORES = 8
BPC = B // NCORES
P = BPC * C                    # 128 partitions
INV_S = 1.0 / float(S)

ACC_W = 2048
ZT = 2048                      # pass-2 PSUM/ACT tile cols (4 banks)
MMT = 512
OBUFS = 7

# Per-phase load chunks: (cols, sink). dve -> adds into acc half;
# act -> in-place Copy+accum_out; dver -> DVE direct reduce.
LOADS0 = [
    (4096, "act"), (4096, "act"), (4096, "dve"), (4096, "dve"),
    (4096, "dve"), (4096, "dve"), (4096, "dve"),
    (2048, "act"), (1024, "dver"), (512, "act"), (512, "dver"),
]
LOADS1 = [
    (4096, "dve"), (4096, "dve"), (4096, "dve"), (4096, "dve"),
    (4096, "dve"), (4096, "dve"), (4096, "dve"),
    (2048, "dver"), (1024, "act"), (512, "dver"), (512, "dver"),
]
assert sum(c for c, _ in LOADS0) == HS
assert sum(c for c, _ in LOADS1) == HS

# pass-2 column plan per phase: (store_cols, [act_tile_cols...])
STORES0 = [(4096, [2048, 2048])] * 8
STORES1 = [(4096, [2048, 2048])] * 7 + [(2048, [2048]), (1024, [1024]), (1024, [1024])]
assert sum(sc for sc, _ in STORES0) == HS
assert sum(sc for sc, _ in STORES1) == HS


def _build_program():
    nc = bacc.Bacc("TRN2", target_bir_lowering=False, debug=False)

    x_d = nc.dram_tensor("x", [P, S], F16, kind="ExternalInput")
    mba_d = nc.dram_tensor("mba", [C, C + 1], BF16, kind="ExternalInput")
    rT_d = nc.dram_tensor("rT", [1, C], F32, kind="ExternalInput")
    phiRow_d = nc.dram_tensor("phiRow", [1, C], BF16, kind="ExternalInput")
    bcol_d = nc.dram_tensor("bcol", [P, 1], F32, kind="ExternalInput")
    eye2_d = nc.dram_tensor("eye2", [P, C], BF16, kind="ExternalInput")
    out_d = nc.dram_tensor("out", [P, S], F16, kind="ExternalOutput")

    X = mybir.AxisListType.X
    Tanh = mybir.ActivationFunctionType.Tanh
    Copy = mybir.ActivationFunctionType.Copy

    with tile.TileContext(nc) as tc:
        with (
            tc.tile_pool(name="consts", bufs=1) as consts,
            tc.tile_pool(name="xret", bufs=1) as rpool,
            tc.tile_pool(name="stats", bufs=1) as stats,
            tc.tile_pool(name="opool", bufs=OBUFS) as opool,
            tc.tile_pool(name="ps", bufs=2, space="PSUM") as ps,
        ):
            # ---- constants to SBUF (SWDGE so SP stays free for x loads) ----
            mba_sb = consts.tile([C, C + 1], BF16, name="mba_sb")
            nc.gpsimd.dma_start(mba_sb[:], mba_d[:])
            rT_sb = consts.tile([1, C], F32, name="rT_sb")
            nc.gpsimd.dma_start(rT_sb[:], rT_d[:])
            phiRow_sb = consts.tile([1, C], BF16, name="phiRow_sb")
            nc.gpsimd.dma_start(phiRow_sb[:], phiRow_d[:])
            bcol_sb = consts.tile([P, 1], F32, name="bcol_sb")
            nc.gpsimd.dma_start(bcol_sb[:], bcol_d[:])
            eye2_sb = consts.tile([P, C], BF16, name="eye2_sb")
            nc.gpsimd.dma_start(eye2_sb[:], eye2_d[:])

            xret = rpool.tile([P, S], F16, name="xret")
            accA = stats.tile([P, ACC_W], F16, name="accA")
            nc.vector.memset(accA[:], 0.0)
            accB = stats.tile([P, ACC_W], F16, name="accB")
            nc.vector.memset(accB[:], 0.0)
            NK = max(len(LOADS0), len(LOADS1)) + 2
            sums_nkA = stats.tile([P, NK], F32, name="sums_nkA")
            sums_nkB = stats.tile([P, NK], F32, name="sums_nkB")

            def emit_loads(base, loads):
                off = base
                for cols, _ in loads:
                    sl = slice(off, off + cols)
                    nc.sync.dma_start(xret[:, sl], x_d[:, sl])
                    off += cols

            def emit_sinks(base, loads, acc, sums_nk):
                nacc = 0
                off = base
                for cols, sink in loads:
                    sl = slice(off, off + cols)
                    if sink == "dve":
                        for j in range(max(1, cols // ACC_W)):
                            g0 = off + j * ACC_W
                            g1 = off + min((j + 1) * ACC_W, cols)
                            nc.vector.tensor_add(
                                acc[:, 0 : g1 - g0], acc[:, 0 : g1 - g0],
                                xret[:, g0:g1],
                            )
                    elif sink == "act":
                        nc.scalar.activation(
                            xret[:, sl], xret[:, sl], Copy,
                            accum_out=sums_nk[:, nacc : nacc + 1],
                        )
                        nacc += 1
                    else:  # dver
                        nc.vector.reduce_sum(
                            sums_nk[:, nacc : nacc + 1], xret[:, sl], X
                        )
                        nacc += 1
                    off += cols
                nc.vector.reduce_sum(sums_nk[:, nacc : nacc + 1], acc[:], X)
                nacc += 1
                return nacc

            def emit_tiny(s, sums_nk, nacc, M2p_s, bias2_s):
                """Fold (h,c) partials over h, then build M2p_s (block-diag
                [P,P]) and bias2_s [P,1] for sample s."""
                sums = stats.tile([P, 1], F32, name=f"sums{s}")
                nc.vector.reduce_sum(sums[:, 0:1], sums_nk[:, 0:nacc], X)
                sums16 = stats.tile([P, 1], BF16, name=f"sums16_{s}")
                nc.vector.tensor_copy(sums16[:], sums[:])
                t = ps.tile([P, ZT], F32, name=f"tt{s}", tag="z")
                # fold over halves: [64,1] channel sums
                cs_ps = t[0:C, 0:1]
                nc.tensor.matmul(cs_ps, eye2_sb[:], sums16[:],
                                 start=True, stop=True)
                cs16 = stats.tile([C, 1], BF16, name=f"cs16_{s}")
                nc.vector.tensor_copy(cs16[:], cs_ps)
                # w row + sigma: [1, C+1] = cs16^T @ mba
                wa = t[0:1, C + 1 : 2 * (C + 1)]
                nc.tensor.matmul(wa, cs16[:], mba_sb[:], start=True, stop=True)
                w2T = stats.tile([1, C], BF16, name=f"w2T{s}")
                nc.vector.tensor_add(w2T[:], wa[0:1, 0:C], rT_sb[:])
                sig = stats.tile([1, 1], BF16, name=f"sig{s}")
                nc.vector.tensor_copy(sig[:], wa[0:1, C : C + 1])
                # block-diag M2p_s and bias2_s, one 64-block per pixel-half
                nc.vector.memset(M2p_s[:], 0.0)
                t2 = ps.tile([P, ZT], F32, name=f"tu{s}", tag="z")
                for h in range(2):
                    R = slice(h * C, (h + 1) * C)
                    blk = t2[R, h * C : (h + 1) * C]
                    nc.tensor.matmul(blk, w2T[:], phiRow_sb[:],
                                     start=True, stop=True,
                                     tile_position=(0, h * C))
                    nc.vector.tensor_add(M2p_s[R, h * C : (h + 1) * C],
                                         blk, eye2_sb[R, :])
                    bb = t2[R, 2 * C + h : 2 * C + h + 1]
                    nc.tensor.matmul(bb, phiRow_sb[:], sig[:],
                                     start=True, stop=True,
                                     tile_position=(0, h * C))
                    nc.vector.tensor_add(bias2_s[R, 0:1], bb, bcol_sb[R, 0:1])

            def emit_pass2(base, stores, M2p_s, bias2_s):
                off = base
                for sc, tiles in stores:
                    ot = opool.tile([P, sc], F16, name="ot", tag="ot")
                    toff = 0
                    for tcols in tiles:
                        z = ps.tile([P, ZT], F32, name="z", tag="z")
                        for j in range(tcols // MMT):
                            g0 = off + toff + j * MMT
                            nc.tensor.matmul(
                                z[:, j * MMT : (j + 1) * MMT], M2p_s[:],
                                xret[:, g0 : g0 + MMT],
                                start=True, stop=True,
                            )
                        nc.scalar.activation(
                            ot[:, toff : toff + tcols], z[:, 0:tcols], Tanh,
                            bias=bias2_s[:, 0:1],
                        )
                        toff += tcols
                    nc.sync.dma_start(out_d[:, off : off + sc], ot[:, 0:sc])
                    off += sc

            M2p0 = stats.tile([P, P], F16, name="M2p0")
            M2p1 = stats.tile([P, P], F16, name="M2p1")
            bias20 = stats.tile([P, 1], F32, name="bias20")
            bias21 = stats.tile([P, 1], F32, name="bias21")

            emit_loads(0, LOADS0)
            naccA = emit_sinks(0, LOADS0, accA, sums_nkA)
            emit_loads(HS, LOADS1)
            emit_tiny(0, sums_nkA, naccA, M2p0, bias20)
            naccB = emit_sinks(HS, LOADS1, accB, sums_nkB)
            emit_pass2(0, STORES0, M2p0, bias20)
            emit_tiny(1, sums_nkB, naccB, M2p1, bias21)
            emit_pass2(HS, STORES1, M2p1, bias21)

    nc.compile()
    return nc


def _host_consts(theta_w, theta_b, g1_w, g1_b, g2_w, g2_b, phi_w, phi_b):
    """Fold the GCN parameter chain into the device-side constant tensors."""
    f8 = np.float64
    theta_w = theta_w.astype(f8)
    theta_b = theta_b.astype(f8)
    g1_w = g1_w.astype(f8)
    g1_b = g1_b.astype(f8)
    g2w = f8(g2_w.reshape(-1)[0])
    g2b = f8(g2_b.reshape(-1)[0])
    phi_w = phi_w.astype(f8)
    phi_b = phi_b.astype(f8)

    A = g2w * (g1_w.T @ theta_w)            # [C, C]
    r = (g2w * g1_b + g2b) @ theta_w        # [C]
    a = g2w * (g1_w.T @ theta_b)            # [C]
    s0 = (g2w * g1_b + g2b) @ theta_b       # scalar

    mba = np.hstack([A * INV_S, (a * INV_S)[:, None]])        # [C, C+1]
    eye2 = np.vstack([np.eye(C), np.eye(C)])                  # [P, C]
    bcol = np.tile(phi_w * s0 + phi_b, BPC)[:, None]

    c32 = lambda t: np.ascontiguousarray(t, dtype=np.float32)
    cb = lambda t: np.ascontiguousarray(t.astype(np.float32)).astype(
        ml_dtypes.bfloat16
    )
    return {
        "mba": cb(mba),
        "rT": c32(r[None, :]),
        "phiRow": cb(phi_w[None, :]),
        "bcol": c32(bcol),
        "eye2": cb(eye2),
    }


_NC_CACHE = {}


def _get_nc():
    if "nc" not in _NC_CACHE:
        _NC_CACHE["nc"] = _build_program()
    return _NC_CACHE["nc"]


def _pack(x16):
    """[8, 2, 64, S] -> [8, 128, S] with partition p=(half,h*64+c) and
    cols = [sample0 half-pixels | sample1 half-pixels]."""
    x5 = x16.reshape(NCORES, 2, C, 2, HS).transpose(0, 1, 3, 2, 4)
    x5 = np.ascontiguousarray(x5).reshape(NCORES, 2, P, HS)
    return np.concatenate([x5[:, 0], x5[:, 1]], axis=2)


def _unpack(o):
    """inverse of _pack: [8, 128, S] -> [8, 2, 64, S]"""
    o5 = np.stack([o[:, :, :HS], o[:, :, HS:]], axis=1)      # [8,2,P,HS]
    o5 = o5.reshape(NCORES, 2, 2, C, HS).transpose(0, 1, 3, 2, 4)
    return np.ascontiguousarray(o5).reshape(NCORES, 2, C, S)


def _run(inputs, trace=False):
    x = np.asarray(inputs["x"])
    consts = _host_consts(
        np.asarray(inputs["theta_w"]), np.asarray(inputs["theta_b"]),
        np.asarray(inputs["g1_w"]), np.asarray(inputs["g1_b"]),
        np.asarray(inputs["g2_w"]), np.asarray(inputs["g2_b"]),
        np.asarray(inputs["phi_w"]), np.asarray(inputs["phi_b"]),
    )
    x16 = np.ascontiguousarray(x, dtype=np.float16).reshape(NCORES, 2, C, S)
    xp = _pack(x16)
    in_maps = [{"x": np.ascontiguousarray(xp[k]), **consts} for k in range(NCORES)]

    nc = _get_nc()
    res = run_bass_kernel_spmd(
        nc, in_maps, core_ids=list(range(NCORES)), trace=trace
    )
    op = np.stack([res.results[k]["out"] for k in range(NCORES)])
    out = _unpack(op).astype(np.float32).reshape(B, C, H, W)
    return out, res


def kernel(**inputs):
    out, _ = _run(inputs, trace=False)
    return out


# revision 4
# speedup vs baseline: 1.1894x; 1.0156x over previous
"""Trainium2 Bass kernel for nn_CAD_GCN (gnn_message_passing), v4.

Math (per core, 2 samples): out = tanh(x + phi_w*y + phi_b) with
y = sum_c w_eff[c] x[c,:] + bias_eff, w_eff = x_node@A + r (see
_host_consts); x_node is the per-sample spatial mean.

v4 layout: the host packs each sample's pixels onto the FULL 128
partitions: partition p = (pixel_half h, channel c), columns =
32768 half-pixels; per-core tensor [128, 65536] = [sample0 | sample1].
The two samples are processed as two phases:
  phase A: stream sample-0's 32768 cols (23 us of DMA), reduce per
    chunk (DVE fp16 adds / ACT copy+accum / DVE direct for tails),
    fold the (h,c) partial sums over h with a tiny PE matmul, build
    M2p0 = blockdiag(outer(w_eff,phi_w)+I, same) and bias2; then run
    sample-0's matmul+tanh+store pass WHILE sample-1's columns are
    still streaming (phase B loads queue right behind phase A's).
  phase B: same for sample 1; its reductions run on DVE under
    sample-0's tanh stream, so ACT (the only tanh engine and the
    critical resource, 0.833ns/col) stays continuously busy.
All compute tiles are full 128-partition width (engine cost is per
free-dim column, so half-height tiles would double engine time).

x is staged fp16 by the host; output written fp16, host upcasts
(rel-err gate 2e-2; measured ~7e-4).
"""

import sys

for _p in ("/opt/trn_rl_repo",):
    if _p not in sys.path:
        sys.path.insert(0, _p)

import numpy as np
import ml_dtypes

import concourse.bacc as bacc
import concourse.mybir as mybir
import concourse.tile as tile
from concourse.bass_utils import run_bass_kernel_spmd

F32 = mybir.dt.float32
F16 = mybir.dt.float16
BF16 = mybir.dt.bfloat16

B, C, H, W = 16, 64, 256, 256
S = H * W                      # pixels per sample
HS = S // 2                    # half-pixels = cols per phase
NCORES = 8
BPC = B // NCORES
P = BPC * C                    # 128 partitions
INV_S = 1.0 / float(S)

ACC_W = 1024
ZT = 2048                      # pass-2 PSUM/ACT tile cols (4 banks)
MMT = 512
OBUFS = 7

# Per-phase load chunks: (cols, sink). dve -> adds into acc half;
# act -> in-place Copy+accum_out; dver -> DVE direct reduce.
LOADS0 = [
    (4096, "act"), (4096, "act"), (4096, "dve"), (4096, "dve"),
    (4096, "dve"), (4096, "dve"), (4096, "dve"), (2048, "dve"),
    (1024, "act"), (512, "act"), (512, "act"),
]
LOADS1 = [
    (4096, "dve"), (4096, "dve"), (4096, "dve"), (4096, "dve"),
    (4096, "dve"), (4096, "dve"), (4096, "dve"), (2048, "dve"),
    (1024, "dver"), (512, "dver"), (512, "dver"),
]
assert sum(c for c, _ in LOADS0) == HS
assert sum(c for c, _ in LOADS1) == HS

# pass-2 column plan per phase: (store_cols, [act_tile_cols...], engine)
STORES0 = [(512, [512], "sync"), (1536, [1536], "sync"),
           (2048, [2048], "sync")] + [(4096, [2048, 2048], "sync")] * 7
STORES1 = [(512, [512], "sync"), (1536, [1536], "sync")] + \
    [(4096, [2048, 2048], "sync")] * 7 + \
    [(1024, [1024], "sync"), (512, [512], "gpsimd"), (512, [512], "sync")]
assert sum(sc for sc, _, _ in STORES0) == HS
assert sum(sc for sc, _, _ in STORES1) == HS
TINY1_AT = 8                   # emit sample-1 tiny stage after this many
                               # phase-A store chunks (PE readiness)


def _build_program():
    nc = bacc.Bacc("TRN2", target_bir_lowering=False, debug=False)

    x_d = nc.dram_tensor("x", [P, S], F16, kind="ExternalInput")
    mba2_d = nc.dram_tensor("mba2", [P, C + 1], BF16, kind="ExternalInput")
    rTx_d = nc.dram_tensor("rTx", [1, C + 1], BF16, kind="ExternalInput")
    phiRow_d = nc.dram_tensor("phiRow", [1, C], BF16, kind="ExternalInput")
    one1_d = nc.dram_tensor("one1", [1, 1], BF16, kind="ExternalInput")
    eyeb_d = nc.dram_tensor("eyeb", [C, C], BF16, kind="ExternalInput")
    ebc_d = nc.dram_tensor("ebc", [C, C + 1], BF16, kind="ExternalInput")
    out_d = nc.dram_tensor("out", [P, S], F16, kind="ExternalOutput")

    X = mybir.AxisListType.X
    Tanh = mybir.ActivationFunctionType.Tanh
    Copy = mybir.ActivationFunctionType.Copy

    with tile.TileContext(nc) as tc:
        with (
            tc.tile_pool(name="consts", bufs=1) as consts,
            tc.tile_pool(name="xret", bufs=1) as rpool,
            tc.tile_pool(name="stats", bufs=1) as stats,
            tc.tile_pool(name="opool", bufs=OBUFS) as opool,
            tc.tile_pool(name="ps", bufs=2, space="PSUM") as ps,
        ):
            # ---- constants to SBUF (SWDGE so SP stays free for x loads) ----
            mba2_sb = consts.tile([P, C + 1], BF16, name="mba2_sb")
            nc.gpsimd.dma_start(mba2_sb[:], mba2_d[:])
            rTx_sb = consts.tile([1, C + 1], BF16, name="rTx_sb")
            nc.gpsimd.dma_start(rTx_sb[:], rTx_d[:])
            phiRow_sb = consts.tile([1, C], BF16, name="phiRow_sb")
            nc.gpsimd.dma_start(phiRow_sb[:], phiRow_d[:])
            one1_sb = consts.tile([1, 1], BF16, name="one1_sb")
            nc.gpsimd.dma_start(one1_sb[:], one1_d[:])
            eyeb_sb = consts.tile([C, C], BF16, name="eyeb_sb")
            nc.gpsimd.dma_start(eyeb_sb[:], eyeb_d[:])
            ebc_sb = consts.tile([C, C + 1], BF16, name="ebc_sb")
            nc.gpsimd.dma_start(ebc_sb[:], ebc_d[:])
            # [phi_w | sigma_s] rows for the fused M2+bias matmul
            phis0 = consts.tile([1, C + 1], BF16, name="phis0")
            nc.vector.tensor_copy(phis0[0:1, 0:C], phiRow_sb[:])
            phis1 = consts.tile([1, C + 1], BF16, name="phis1")
            nc.vector.tensor_copy(phis1[0:1, 0:C], phiRow_sb[:])

            xret = rpool.tile([P, S], F16, name="xret")
            accA = stats.tile([P, ACC_W], F16, name="accA")
            nc.vector.memset(accA[:], 0.0)
            accB = stats.tile([P, ACC_W], F16, name="accB")
            nc.vector.memset(accB[:], 0.0)
            NK = max(len(LOADS0), len(LOADS1)) + 2
            sums_nkA = stats.tile([P, NK], F32, name="sums_nkA")
            sums_nkB = stats.tile([P, NK], F32, name="sums_nkB")

            def emit_loads(base, loads):
                off = base
                for cols, _ in loads:
                    sl = slice(off, off + cols)
                    nc.sync.dma_start(xret[:, sl], x_d[:, sl])
                    off += cols

            def emit_sinks(base, loads, acc, sums_nk):
                nacc = 0
                off = base
                last_dve = max(i for i, (_, s) in enumerate(loads) if s == "dve")
                for ci, (cols, sink) in enumerate(loads):
                    sl = slice(off, off + cols)
                    if sink == "dve":
                        for j in range(max(1, cols // ACC_W)):
                            g0 = off + j * ACC_W
                            g1 = off + min((j + 1) * ACC_W, cols)
                            nc.vector.tensor_add(
                                acc[:, 0 : g1 - g0], acc[:, 0 : g1 - g0],
                                xret[:, g0:g1],
                            )
                    elif sink == "act":
                        nc.scalar.activation(
                            xret[:, sl], xret[:, sl], Copy,
                            accum_out=sums_nk[:, nacc : nacc + 1],
                        )
                        nacc += 1
                    else:  # dver
                        nc.vector.reduce_sum(
                            sums_nk[:, nacc : nacc + 1], xret[:, sl], X
                        )
                        nacc += 1
                    if ci == last_dve:
                        nc.vector.reduce_sum(
                            sums_nk[:, nacc : nacc + 1], acc[:], X
                        )
                        nacc += 1
                    off += cols
                return nacc

            def emit_tiny(s, sums_nk, nacc, M2p_s, bias2_s, phis_s):
                """Combine partials (the vstacked mba2 folds the pixel-halves
                inside the matmul) with rT accumulated via a ones-row matmul,
                then per half one accumulating matmul pair builds
                [M2-block + I | bias + bcol] in PSUM; ACT copies move the
                results to SBUF. Only combine+copy touch DVE (no PE-dependent
                DVE ops, so phase-B reduce adds cannot delay this chain)."""
                sums = stats.tile([P, 1], F32, name=f"sums{s}")
                nc.vector.reduce_sum(sums[:, 0:1], sums_nk[:, 0:nacc], X)
                sums16 = stats.tile([P, 1], BF16, name=f"sums16_{s}")
                nc.vector.tensor_copy(sums16[:], sums[:])
                t = ps.tile([P, ZT], F32, name=f"tt{s}", tag="z")
                # w row + sigma (+rT): [1, C+1] = sums16^T @ mba2 + 1 @ rTx
                wa = t[0:1, 0 : C + 1]
                nc.tensor.matmul(wa, sums16[:], mba2_sb[:], start=True, stop=False)
                nc.tensor.matmul(wa, one1_sb[:], rTx_sb[:], start=False, stop=True)
                w2Te = stats.tile([1, C + 1], BF16, name=f"w2Te{s}")
                nc.scalar.copy(w2Te[:], wa)
                nc.scalar.copy(phis_s[0:1, C : C + 1], wa[0:1, C : C + 1])
                # per half: [64, C+1] = w2T^T-row outer [phi|sigma] + [I|bcol]
                t2 = ps.tile([P, ZT], F32, name=f"tu{s}", tag="z")
                for h in range(2):
                    R = slice(h * C, (h + 1) * C)
                    c0 = h * (C + 1)
                    nc.tensor.matmul(t2[R, c0 : c0 + C + 1],
                                     w2Te[0:1, 0:C], phis_s[:],
                                     start=True, stop=False,
                                     tile_position=(0, h * C))
                    nc.tensor.matmul(t2[R, c0 : c0 + C + 1],
                                     eyeb_sb[:], ebc_sb[:],
                                     start=False, stop=True,
                                     tile_position=(0, h * C))
                    nc.scalar.copy(M2p_s[R, h * C : (h + 1) * C],
                                   t2[R, c0 : c0 + C])
                    nc.scalar.copy(bias2_s[R, 0:1],
                                   t2[R, c0 + C : c0 + C + 1])

            def emit_pass2(base, stores, M2p_s, bias2_s, lo=0, hi=None):
                off = base + sum(sc for sc, _, _ in stores[:lo])
                for sc, tiles, eng in stores[lo:hi]:
                    ot = opool.tile([P, sc], F16, name="ot", tag="ot")
                    toff = 0
                    for tcols in tiles:
                        z = ps.tile([P, ZT], F32, name="z", tag="z")
                        for j in range((tcols + MMT - 1) // MMT):
                            g0 = off + toff + j * MMT
                            mw = min(MMT, tcols - j * MMT)
                            nc.tensor.matmul(
                                z[:, j * MMT : j * MMT + mw], M2p_s[:],
                                xret[:, g0 : g0 + mw],
                                start=True, stop=True,
                            )
                        nc.scalar.activation(
                            ot[:, toff : toff + tcols], z[:, 0:tcols], Tanh,
                            bias=bias2_s[:, 0:1],
                        )
                        toff += tcols
                    getattr(nc, eng).dma_start(
                        out_d[:, off : off + sc], ot[:, 0:sc]
                    )
                    off += sc

            M2p0 = stats.tile([P, P], F16, name="M2p0")
            nc.vector.memset(M2p0[:], 0.0)
            M2p1 = stats.tile([P, P], F16, name="M2p1")
            nc.vector.memset(M2p1[:], 0.0)
            bias20 = stats.tile([P, 1], F32, name="bias20")
            bias21 = stats.tile([P, 1], F32, name="bias21")

            emit_loads(0, LOADS0)
            naccA = emit_sinks(0, LOADS0, accA, sums_nkA)
            emit_loads(HS, LOADS1)
            emit_tiny(0, sums_nkA, naccA, M2p0, bias20, phis0)
            naccB = emit_sinks(HS, LOADS1, accB, sums_nkB)
            emit_pass2(0, STORES0, M2p0, bias20, hi=TINY1_AT)
            emit_tiny(1, sums_nkB, naccB, M2p1, bias21, phis1)
            emit_pass2(0, STORES0, M2p0, bias20, lo=TINY1_AT)
            emit_pass2(HS, STORES1, M2p1, bias21)

    nc.compile()
    return nc


def _host_consts(theta_w, theta_b, g1_w, g1_b, g2_w, g2_b, phi_w, phi_b):
    """Fold the GCN parameter chain into the device-side constant tensors."""
    f8 = np.float64
    theta_w = theta_w.astype(f8)
    theta_b = theta_b.astype(f8)
    g1_w = g1_w.astype(f8)
    g1_b = g1_b.astype(f8)
    g2w = f8(g2_w.reshape(-1)[0])
    g2b = f8(g2_b.reshape(-1)[0])
    phi_w = phi_w.astype(f8)
    phi_b = phi_b.astype(f8)

    A = g2w * (g1_w.T @ theta_w)            # [C, C]
    r = (g2w * g1_b + g2b) @ theta_w        # [C]
    a = g2w * (g1_w.T @ theta_b)            # [C]
    s0 = (g2w * g1_b + g2b) @ theta_b       # scalar

    mba = np.hstack([A * INV_S, (a * INV_S)[:, None]])        # [C, C+1]
    mba2 = np.vstack([mba, mba])                              # [P, C+1]
    eye2 = np.vstack([np.eye(C), np.eye(C)])                  # [P, C]
    bcol = np.tile(phi_w * s0 + phi_b, BPC)[:, None]

    c32 = lambda t: np.ascontiguousarray(t, dtype=np.float32)
    cb = lambda t: np.ascontiguousarray(t.astype(np.float32)).astype(
        ml_dtypes.bfloat16
    )
    rTx = np.hstack([r, [0.0]])[None, :]                      # [1, C+1]
    bcol64 = (phi_w * s0 + phi_b)[:, None]                    # [C, 1]
    ebc = np.hstack([np.eye(C), bcol64])                      # [C, C+1]
    return {
        "mba2": cb(mba2),
        "rTx": cb(rTx),
        "phiRow": cb(phi_w[None, :]),
        "one1": cb(np.ones((1, 1))),
        "eyeb": cb(np.eye(C)),
        "ebc": cb(ebc),
    }


_NC_CACHE = {}


def _get_nc():
    if "nc" not in _NC_CACHE:
        _NC_CACHE["nc"] = _build_program()
    return _NC_CACHE["nc"]


def _pack(x16):
    """[8, 2, 64, S] -> [8, 128, S] with partition p=(half,h*64+c) and
    cols = [sample0 half-pixels | sample1 half-pixels]."""
    x5 = x16.reshape(NCORES, 2, C, 2, HS).transpose(0, 1, 3, 2, 4)
    x5 = np.ascontiguousarray(x5).reshape(NCORES, 2, P, HS)
    return np.concatenate([x5[:, 0], x5[:, 1]], axis=2)


def _unpack(o):
    """inverse of _pack: [8, 128, S] -> [8, 2, 64, S]"""
    o5 = np.stack([o[:, :, :HS], o[:, :, HS:]], axis=1)      # [8,2,P,HS]
    o5 = o5.reshape(NCORES, 2, 2, C, HS).transpose(0, 1, 3, 2, 4)
    return np.ascontiguousarray(o5).reshape(NCORES, 2, C, S)


def _run(inputs, trace=False):
    x = np.asarray(inputs["x"])
    consts = _host_consts(
        np.asarray(inputs["theta_w"]), np.asarray(inputs["theta_b"]),
        np.asarray(inputs["g1_w"]), np.asarray(inputs["g1_b"]),
        np.asarray(inputs["g2_w"]), np.asarray(inputs["g2_b"]),
        np.asarray(inputs["phi_w"]), np.asarray(inputs["phi_b"]),
    )
    x16 = np.ascontiguousarray(x, dtype=np.float16).reshape(NCORES, 2, C, S)
    xp = _pack(x16)
    in_maps = [{"x": np.ascontiguousarray(xp[k]), **consts} for k in range(NCORES)]

    nc = _get_nc()
    res = run_bass_kernel_spmd(
        nc, in_maps, core_ids=list(range(NCORES)), trace=trace
    )
    op = np.stack([res.results[k]["out"] for k in range(NCORES)])
    out = _unpack(op).astype(np.float32).reshape(B, C, H, W)
    return out, res


def kernel(**inputs):
    out, _ = _run(inputs, trace=False)
    return out
